# revision 15
# baseline (speedup 1.0000x reference)
"""Trainium2 Bass kernel for nn_AxisNetFusion (ChebConv GNN + PAE edge encoder).

Strategy (8 NeuronCores, node/destination-partitioned graph parallel):
  Launch 1 (edge MLP): edges split evenly across cores; each core runs the
    PAE parser (two matmul stacks sharing weights) on its edge slice and
    emits the cosine edge weights.
  Host: degree / symmetric normalization (O(E) scalar math) + builds the
    col-sorted scatter structure: per-core per-destination-block one-hot
    matrices scaled by `norm` (so the scatter is a plain PSUM-accumulated
    matmul) and int16 gather indices for the per-edge x[row] rows.
  Launch 2 (4 ChebConv layers + JK + cls head): each core owns 2560
    destination rows. Per propagation: dma_gather of x[row] rows, scatter
    via one-hot matmuls into PSUM, AllGather of the updated node slices
    between cores. Layer update h = relu(sum_k Txk @ Wk) runs in a
    transposed layout fed by PE transposes of the prop outputs.
"""

import os
import sys

sys.path.insert(0, "/opt/trn_rl_repo")

import numpy as np
from concourse import bass, bacc, mybir, tile
from concourse import bass_utils
from concourse.masks import make_identity

F32 = mybir.dt.float32
I16 = mybir.dt.int16
AF = mybir.ActivationFunctionType
OP = mybir.AluOpType

# Problem sizes (fixed by the task spec).
N = 20000
E = 400000
D = 256
L = 4
EIN = 32
EH = 128
NCLS = 2
JK = L * D
BN_EPS = 1e-5

NCORES = 8
NP = 20480            # padded node count (8 * 2560)
R = NP // NCORES      # rows (destination nodes) per core
NB = R // 128         # 128-row destination blocks per core

EPC = 50176           # padded edges per core for the edge MLP (98 * 512)
CHA = EPC // 512      # edge-MLP chunks of 512 edges
EPAD = NCORES * EPC

_CACHE: dict = {}
LAST_EXEC_NS: dict = {}
LAST_RESULTS: dict = {}


# --------------------------------------------------------------------------
# Launch 1: PAE edge encoder -> edge weights
# --------------------------------------------------------------------------
def build_edge_kernel():
    nc = bacc.Bacc(trn_type="TRN2", num_devices=NCORES)
    xT = nc.dram_tensor("xT", [2 * EIN, EPC], F32, kind="ExternalInput").ap()
    w1 = nc.dram_tensor("w1", [EIN, EH], F32, kind="ExternalInput").ap()
    w2p = nc.dram_tensor("w2p", [EH, EH], F32, kind="ExternalInput").ap()
    b1 = nc.dram_tensor("b1", [EH, 1], F32, kind="ExternalInput").ap()
    b2p = nc.dram_tensor("b2p", [EH, 1], F32, kind="ExternalInput").ap()
    tones = nc.dram_tensor("tones", [128, 256], F32, kind="ExternalInput").ap()
    ew = nc.dram_tensor("ew", [CHA, 512], F32, kind="ExternalOutput").ap()

    with tile.TileContext(nc) as tc:
        with (
            tc.tile_pool(name="const", bufs=1) as cp,
            tc.tile_pool(name="sb", bufs=3) as sb,
            tc.tile_pool(name="ps", bufs=2, space="PSUM") as ps,
            tc.tile_pool(name="pss", bufs=1, space="PSUM") as pss,
        ):
            w1_sb = cp.tile([EIN, EH], F32)
            nc.sync.dma_start(out=w1_sb[:], in_=w1[:, :])
            w2p_sb = cp.tile([EH, EH], F32)
            nc.sync.dma_start(out=w2p_sb[:], in_=w2p[:, :])
            b1_sb = cp.tile([EH, 1], F32)
            nc.sync.dma_start(out=b1_sb[:], in_=b1[:, :])
            b2p_sb = cp.tile([EH, 1], F32)
            nc.sync.dma_start(out=b2p_sb[:], in_=b2p[:, :])
            tones_sb = cp.tile([128, 256], F32)
            nc.sync.dma_start(out=tones_sb[:], in_=tones[:, :])

            # Per-edge gram terms accumulate here: rows = chunk id,
            # cols = [s11 | s12 | s22] x 512 edges.
            psS = pss.tile([128, 1536], F32, space="PSUM")

            for c in range(CHA):
                xta = sb.tile([EIN, 512], F32, tag="xta")
                nc.sync.dma_start(out=xta[:], in_=xT[0:EIN, c * 512:(c + 1) * 512])
                xtb = sb.tile([EIN, 512], F32, tag="xtb")
                nc.sync.dma_start(out=xtb[:],
                                  in_=xT[EIN:2 * EIN, c * 512:(c + 1) * 512])

                pm1 = ps.tile([128, 512], F32, tag="pm1", space="PSUM")
                nc.tensor.matmul(out=pm1[:], lhsT=w1_sb[:], rhs=xta[:],
                                 start=True, stop=True)
                r1re = sb.tile([128, 512], F32, tag="r1re")
                nc.scalar.activation(r1re[:], pm1[:], AF.Relu, bias=b1_sb[:, 0:1])

                pm1b = ps.tile([128, 512], F32, tag="pm1", space="PSUM")
                nc.tensor.matmul(out=pm1b[:], lhsT=w1_sb[:], rhs=xtb[:],
                                 start=True, stop=True)
                r2re = sb.tile([128, 512], F32, tag="r2re")
                nc.scalar.activation(r2re[:], pm1b[:], AF.Relu, bias=b1_sb[:, 0:1])

                pm2 = ps.tile([128, 512], F32, tag="pm2", space="PSUM")
                nc.tensor.matmul(out=pm2[:], lhsT=w2p_sb[:], rhs=r1re[:],
                                 start=True, stop=True)
                r1 = sb.tile([128, 512], F32, tag="r1")
                nc.vector.tensor_scalar(r1[:], pm2[:], b2p_sb[:, 0:1], None, op0=OP.add)

                pm2b = ps.tile([128, 512], F32, tag="pm2", space="PSUM")
                nc.tensor.matmul(out=pm2b[:], lhsT=w2p_sb[:], rhs=r2re[:],
                                 start=True, stop=True)
                r2 = sb.tile([128, 512], F32, tag="r2")
                nc.vector.tensor_scalar(r2[:], pm2b[:], b2p_sb[:, 0:1], None, op0=OP.add)

                prod = sb.tile([128, 1536], F32, tag="prod")
                nc.vector.tensor_tensor(prod[:, 0:512], r1[:], r1[:], op=OP.mult)
                nc.vector.tensor_tensor(prod[:, 512:1024], r1[:], r2[:], op=OP.mult)
                nc.vector.tensor_tensor(prod[:, 1024:1536], r2[:], r2[:], op=OP.mult)

                # Column-reduce (over hidden dim = partitions) into row c of psS
                # via a ones-column matmul; the sliding window of `tones` puts
                # the all-ones column at local column c.
                for j in range(3):
                    nc.tensor.matmul(
                        out=psS[:, j * 512:(j + 1) * 512],
                        lhsT=tones_sb[:, 128 - c:256 - c],
                        rhs=prod[:, j * 512:(j + 1) * 512],
                        start=(c == 0), stop=(c == CHA - 1),
                        skip_group_check=True,
                    )

            # cos = s12 / (max(sqrt(s11),eps) * max(sqrt(s22),eps));
            # ew = (cos+1)/2, all on [chunks x 512] full-width tiles.
            m11 = sb.tile([128, 512], F32, tag="m11")
            nc.vector.tensor_scalar(m11[:], psS[:, 0:512], 1e-16, None, op0=OP.max)
            m22 = sb.tile([128, 512], F32, tag="m22")
            nc.vector.tensor_scalar(m22[:], psS[:, 1024:1536], 1e-16, None, op0=OP.max)
            pm = sb.tile([128, 512], F32, tag="pm")
            nc.vector.tensor_tensor(pm[:], m11[:], m22[:], op=OP.mult)
            sq = sb.tile([128, 512], F32, tag="sq")
            nc.scalar.activation(sq[:], pm[:], AF.Sqrt)
            inv = sb.tile([128, 512], F32, tag="inv")
            nc.vector.reciprocal(inv[:], sq[:])
            cosv = sb.tile([128, 512], F32, tag="cosv")
            nc.vector.tensor_tensor(cosv[:], psS[:, 512:1024], inv[:], op=OP.mult)
            ewt = sb.tile([128, 512], F32, tag="ewt")
            nc.scalar.activation(ewt[:], cosv[:], AF.Copy, bias=0.5, scale=0.5)
            nc.sync.dma_start(out=ew[:, :], in_=ewt[0:CHA, :])

    nc.compile()
    return nc


# --------------------------------------------------------------------------
# Launch 2: 4x ChebConv + JK concat + cls head
# --------------------------------------------------------------------------
def build_gnn_kernel(CPB):
    TOTCH = NB * CPB
    nc = bacc.Bacc(trn_type="TRN2", num_devices=NCORES)
    xfull0 = nc.dram_tensor("xfull0", [NP, D], F32, kind="ExternalInput").ap()
    xown = nc.dram_tensor("xown", [R, D], F32, kind="ExternalInput").ap()
    gidx = nc.dram_tensor("gidx", [128, TOTCH * 8], I16, kind="ExternalInput").ap()
    lhsTd = nc.dram_tensor("lhsTd", [128, TOTCH * 128], F32, kind="ExternalInput").ap()
    chebd = nc.dram_tensor("chebd", [L * 3 * D, D], F32, kind="ExternalInput").ap()
    clsw1d = nc.dram_tensor("clsw1d", [JK, D], F32, kind="ExternalInput").ap()
    clsb1d = nc.dram_tensor("clsb1d", [D, 1], F32, kind="ExternalInput").ap()
    clsw2d = nc.dram_tensor("clsw2d", [D, NCLS], F32, kind="ExternalInput").ap()
    clsb2d = nc.dram_tensor("clsb2d", [NCLS, 1], F32, kind="ExternalInput").ap()
    logitT = nc.dram_tensor("logitT", [NCLS, R], F32, kind="ExternalOutput").ap()

    RG = [list(range(NCORES))]

    with tile.TileContext(nc) as tc:
        with (
            tc.tile_pool(name="const", bufs=1) as cp,
            tc.tile_pool(name="big", bufs=1) as bigp,
            tc.tile_pool(name="gp", bufs=2) as gp,
            tc.tile_pool(name="lp", bufs=2) as lp,
            tc.tile_pool(name="wp", bufs=8) as wp,
            tc.tile_pool(name="sm", bufs=3) as sm,
            tc.tile_pool(name="ps_scat", bufs=2, space="PSUM") as ps_scat,
            tc.tile_pool(name="ps_tp", bufs=2, space="PSUM") as ps_tp,
            tc.tile_pool(name="ps_acc", bufs=2, space="PSUM") as ps_acc,
            tc.tile_pool(name="dram", bufs=1, space="DRAM") as drp,
        ):
            # ---------------- constants / persistent tiles ----------------
            gidx_sb = cp.tile([128, TOTCH * 8], I16)
            nc.sync.dma_start(out=gidx_sb[:], in_=gidx[:, :])
            ident = cp.tile([128, 128], F32)
            make_identity(nc, ident[:])
            clsb1_sb = []
            clsw2_sb = []
            for mc in range(2):
                bt = cp.tile([128, 1], F32, tag=f"clsb1_{mc}")
                nc.sync.dma_start(out=bt[:], in_=clsb1d[mc * 128:(mc + 1) * 128, :])
                clsb1_sb.append(bt)
                wt2 = cp.tile([128, NCLS], F32, tag=f"clsw2_{mc}")
                nc.sync.dma_start(out=wt2[:], in_=clsw2d[mc * 128:(mc + 1) * 128, :])
                clsw2_sb.append(wt2)
            clsb2_sb = cp.tile([NCLS, 1], F32)
            nc.sync.dma_start(out=clsb2_sb[:], in_=clsb2d[:, :])

            # 8 big [128, R] tiles: transposed Tx / h accumulators.
            bigt = [bigp.tile([128, R], F32, tag=f"big{i}", name=f"big{i}")
                    for i in range(8)]
            tx0T = [bigt[0], bigt[1]]
            tx1T = [bigt[2], bigt[3]]
            tx2T = [bigt[4], bigt[5]]
            hT = [bigt[6], bigt[7]]

            # internal DRAM
            tx1slice = drp.tile([R, D], F32)
            hslice = drp.tile([R, D], F32)
            tx1full = [drp.tile([NP, D], F32, addr_space="Shared",
                                name=f"tx1full{l}", tag=f"tx1full{l}")
                       for l in range(L)]
            xf = [drp.tile([NP, D], F32, addr_space="Shared",
                           name=f"xf{l}", tag=f"xf{l}")
                  for l in range(L - 1)]
            jkTd = drp.tile([2 * D, R], F32)  # hT of layers 0 and 1

            def scatter_blocks(xsrc_ap, post):
                """One propagation: for each destination block, gather x[row]
                rows and PSUM-accumulate one-hot matmuls; `post(b, pscat)`
                consumes the [128, D] PSUM block."""
                GSUB = 8  # <=1024 indices per dma_gather (HW packet limit)
                for b in range(NB):
                    g = gp.tile([128, CPB, D], F32, tag="g")
                    for c0 in range(0, CPB, GSUB):
                        c1 = min(c0 + GSUB, CPB)
                        nc.gpsimd.dma_gather(
                            out_ap=g[:, c0:c1, :],
                            in_ap=xsrc_ap,
                            idxs_ap=gidx_sb[:, (b * CPB + c0) * 8:(b * CPB + c1) * 8],
                            num_idxs=(c1 - c0) * 128,
                            num_idxs_reg=(c1 - c0) * 128,
                            elem_size=D,
                        )
                    lt = lp.tile([128, CPB * 128], F32, tag="lt")
                    nc.sync.dma_start(
                        out=lt[:],
                        in_=lhsTd[:, b * CPB * 128:(b + 1) * CPB * 128])
                    pscat = ps_scat.tile([128, D], F32, tag="scat", space="PSUM")
                    for cc in range(CPB):
                        nc.tensor.matmul(
                            out=pscat[:],
                            lhsT=lt[:, cc * 128:(cc + 1) * 128],
                            rhs=g[:, cc, :],
                            start=(cc == 0), stop=(cc == CPB - 1),
                            skip_group_check=True,
                        )
                    post(b, pscat)

            def transpose_into(src_sb, dstT, b):
                # src_sb [128, D] node-major block b -> dstT pair [128, R] cols
                for kc in range(2):
                    ptp = ps_tp.tile([128, 128], F32, tag="tp", space="PSUM")
                    nc.tensor.transpose(ptp[:], src_sb[:, kc * 128:(kc + 1) * 128],
                                        ident[:])
                    nc.vector.tensor_copy(dstT[kc][:, b * 128:(b + 1) * 128], ptp[:])

            for l in range(L):
                xsrc = xfull0[:, :] if l == 0 else xf[l - 1][:, :]
                agh_dst = xf[l] if l < L - 1 else None

                # ---- prop1: Tx1 = S x ----
                with nc.named_scope(f"prop1_l{l}"):
                    def post1(b, pscat):
                        t1 = sm.tile([128, D], F32, tag="t1b")
                        nc.scalar.activation(t1[:], pscat[:], AF.Copy)
                        nc.sync.dma_start(
                            out=tx1slice[b * 128:(b + 1) * 128, :], in_=t1[:])
                        transpose_into(t1, tx1T, b)
                    scatter_blocks(xsrc, post1)

                with nc.named_scope(f"ag_tx1_l{l}"):
                    nc.gpsimd.collective_compute(
                        "AllGather", OP.bypass, replica_groups=RG,
                        ins=[tx1slice.opt()], outs=[tx1full[l].opt()])

                # ---- prop2: Tx2 = 2 S Tx1 - Tx0 ----
                with nc.named_scope(f"prop2_l{l}"):
                    def post2(b, pscat):
                        t0b = sm.tile([128, D], F32, tag="t0b")
                        if l == 0:
                            nc.sync.dma_start(
                                out=t0b[:], in_=xown[b * 128:(b + 1) * 128, :])
                        else:
                            nc.sync.dma_start(
                                out=t0b[:], in_=hslice[b * 128:(b + 1) * 128, :])
                        t2a = sm.tile([128, D], F32, tag="t2a")
                        nc.scalar.activation(t2a[:], pscat[:], AF.Copy, scale=2.0)
                        t2 = sm.tile([128, D], F32, tag="t2b")
                        nc.vector.tensor_tensor(t2[:], t2a[:], t0b[:], op=OP.subtract)
                        transpose_into(t2, tx2T, b)
                        if l == 0:
                            transpose_into(t0b, tx0T, b)
                    scatter_blocks(tx1full[l][:, :], post2)

                # ---- layer update: hT = relu(sum_k Wk^T TxkT) ----
                with nc.named_scope(f"hmm_l{l}"):
                    wt = {}
                    for k in range(3):
                        for kc in range(2):
                            w = wp.tile([128, D], F32, tag="wt")
                            row0 = ((l * 3 + k) * 2 + kc) * 128
                            nc.sync.dma_start(out=w[:], in_=chebd[row0:row0 + 128, :])
                            wt[(k, kc)] = w
                    txTs = [tx0T, tx1T, tx2T]
                    for mc in range(2):
                        for nn in range(5):
                            pacc = ps_acc.tile([128, 512], F32, tag="acc",
                                               space="PSUM")
                            for k in range(3):
                                for kc in range(2):
                                    nc.tensor.matmul(
                                        out=pacc[:],
                                        lhsT=wt[(k, kc)][:, mc * 128:(mc + 1) * 128],
                                        rhs=txTs[k][kc][:, nn * 512:(nn + 1) * 512],
                                        start=(k == 0 and kc == 0),
                                        stop=(k == 2 and kc == 1),
                                        skip_group_check=True,
                                    )
                            nc.scalar.activation(
                                hT[mc][:, nn * 512:(nn + 1) * 512], pacc[:], AF.Relu)

                    if l <= 1:
                        for mc in range(2):
                            nc.sync.dma_start(
                                out=jkTd[(l * 2 + mc) * 128:(l * 2 + mc + 1) * 128, :],
                                in_=hT[mc][:, :])

                # ---- transpose h back, store slice, AllGather ----
                if l < L - 1:
                    with nc.named_scope(f"hback_l{l}"):
                        for b in range(NB):
                            hb = sm.tile([128, D], F32, tag="hb")
                            for kc in range(2):
                                ptp = ps_tp.tile([128, 128], F32, tag="tp",
                                                 space="PSUM")
                                nc.tensor.transpose(
                                    ptp[:], hT[kc][:, b * 128:(b + 1) * 128], ident[:])
                                nc.vector.tensor_copy(
                                    hb[:, kc * 128:(kc + 1) * 128], ptp[:])
                            nc.sync.dma_start(
                                out=hslice[b * 128:(b + 1) * 128, :], in_=hb[:])
                        nc.gpsimd.collective_compute(
                            "AllGather", OP.bypass, replica_groups=RG,
                            ins=[hslice.opt()], outs=[agh_dst.opt()])

                    tx0T, hT = hT, tx0T

            # ---------------- cls head ----------------
            with nc.named_scope("cls"):
                # jk rhs tiles: layers 0/1 from DRAM, 2/3 still in SBUF
                jk_rhs = []
                for j in range(4):
                    dst = [tx1T, tx2T][j // 2][j % 2]
                    nc.sync.dma_start(out=dst[:, :], in_=jkTd[j * 128:(j + 1) * 128, :])
                    jk_rhs.append(dst)
                jk_rhs += [tx0T[0], tx0T[1], hT[0], hT[1]]

                cw = []
                for j in range(8):
                    w = wp.tile([128, D], F32, tag="cw")
                    nc.sync.dma_start(out=w[:], in_=clsw1d[j * 128:(j + 1) * 128, :])
                    cw.append(w)

                for nn in range(5):
                    plog = ps_tp.tile([NCLS, 512], F32, tag="lg", space="PSUM")
                    for mc in range(2):
                        pz = ps_acc.tile([128, 512], F32, tag="acc", space="PSUM")
                        for j in range(8):
                            nc.tensor.matmul(
                                out=pz[:],
                                lhsT=cw[j][:, mc * 128:(mc + 1) * 128],
                                rhs=jk_rhs[j][:, nn * 512:(nn + 1) * 512],
                                start=(j == 0), stop=(j == 7),
                                skip_group_check=True,
                            )
                        zr = sm.tile([128, 512], F32, tag="zr")
                        nc.scalar.activation(
                            zr[:], pz[:], AF.Relu, bias=clsb1_sb[mc][:, 0:1])
                        nc.tensor.matmul(
                            out=plog[:], lhsT=clsw2_sb[mc][:, :],
                            rhs=zr[:], start=(mc == 0), stop=(mc == 1),
                            skip_group_check=True,
                        )
                    lg = sm.tile([NCLS, 512], F32, tag="lgs")
                    nc.vector.tensor_scalar(
                        lg[:], plog[:], clsb2_sb[:, 0:1], None, op0=OP.add)
                    nc.sync.dma_start(
                        out=logitT[:, nn * 512:(nn + 1) * 512], in_=lg[:])

    nc.compile()
    return nc


# --------------------------------------------------------------------------
# Host orchestration
# --------------------------------------------------------------------------
def _wrap_idx16(vals):
    """[n] -> [128, n//16] int16 in the SWDGE wrap layout (16-partition wrap,
    replicated to all 8 Q7 partition groups)."""
    n = vals.shape[0]
    m = np.zeros((16, n // 16), np.int16)
    m[np.arange(n) % 16, np.arange(n) // 16] = vals
    return np.tile(m, (8, 1))


def _prep_edge_inputs(edgenet_input, en_w1, en_b1, en_g1, en_be1, en_w2, en_b2):
    g1k = (en_g1 / np.sqrt(np.float32(1.0 + BN_EPS))).astype(np.float32)
    w2p = (g1k[:, None] * en_w2).astype(np.float32)
    b2p = (en_be1 @ en_w2 + en_b2).astype(np.float32)
    xpad = np.zeros((EPAD, 2 * EIN), np.float32)
    xpad[:E] = edgenet_input
    tones = np.zeros((128, 256), np.float32)
    tones[:, 128] = 1.0
    in_maps = []
    for c in range(NCORES):
        xT = np.ascontiguousarray(xpad[c * EPC:(c + 1) * EPC].T)
        in_maps.append({
            "xT": xT,
            "w1": np.ascontiguousarray(en_w1),
            "w2p": w2p,
            "b1": en_b1.reshape(EH, 1).astype(np.float32),
            "b2p": b2p.reshape(EH, 1),
            "tones": tones,
        })
    return in_maps


def _prep_gnn_inputs(features, row, col, norm, cheb_w,
                     cls_w1, cls_b1, cls_g, cls_b, cls_w2, cls_b2):
    xfull0 = np.zeros((NP, D), np.float32)
    xfull0[:N] = features

    order = np.argsort(col, kind="stable")
    rs, cs, ns = row[order], col[order], norm[order].astype(np.float32)

    # per (core, block) edge counts
    blk = cs // 128                      # global block id, < 160
    counts = np.bincount(blk, minlength=NCORES * NB)
    CPB = max(1, int(np.ceil(counts.max() / 128)))
    TOTCH = NB * CPB
    starts = np.zeros(NCORES * NB + 1, np.int64)
    np.cumsum(counts, out=starts[1:])

    gk = (cls_g / np.sqrt(np.float32(1.0 + BN_EPS))).astype(np.float32)
    clsw2p = (gk[:, None] * cls_w2).astype(np.float32)
    clsb2p = (cls_b @ cls_w2 + cls_b2).astype(np.float32).reshape(NCLS, 1)
    chebd = np.ascontiguousarray(cheb_w.reshape(L * 3 * D, D))
    clsw1 = np.ascontiguousarray(cls_w1)
    clsb1 = cls_b1.reshape(D, 1).astype(np.float32)

    in_maps = []
    for c in range(NCORES):
        gv = np.zeros(TOTCH * 128, np.int64)          # gather row ids
        lh = np.zeros((128, TOTCH * 128), np.float32)  # scaled one-hot lhsT
        for b in range(NB):
            gb = c * NB + b
            s, e = starts[gb], starts[gb + 1]
            cnt = e - s
            slot = np.arange(cnt)
            t = b * CPB + slot // 128                 # chunk id within core
            k = slot % 128                            # edge lane
            gv[t * 128 + k] = rs[s:e]
            lh[k, t * 128 + (cs[s:e] - (c * R + b * 128))] = ns[s:e]
        in_maps.append({
            "xfull0": xfull0,
            "xown": np.ascontiguousarray(xfull0[c * R:(c + 1) * R]),
            "gidx": _wrap_idx16(gv.astype(np.int16)),
            "lhsTd": lh,
            "chebd": chebd,
            "clsw1d": clsw1,
            "clsb1d": clsb1,
            "clsw2d": clsw2p,
            "clsb2d": clsb2p,
        })
    return in_maps, CPB


def kernel(features, edge_index, edgenet_input, cheb_w,
           en_w1, en_b1, en_g1, en_be1, en_w2, en_b2,
           cls_w1, cls_b1, cls_g, cls_b, cls_w2, cls_b2):
    features = np.asarray(features, np.float32)
    edge_index = np.asarray(edge_index)
    edgenet_input = np.asarray(edgenet_input, np.float32)
    cheb_w = np.asarray(cheb_w, np.float32)
    en_w1, en_b1, en_g1, en_be1, en_w2, en_b2 = [
        np.asarray(a, np.float32) for a in (en_w1, en_b1, en_g1, en_be1, en_w2, en_b2)]
    cls_w1, cls_b1, cls_g, cls_b, cls_w2, cls_b2 = [
        np.asarray(a, np.float32) for a in (cls_w1, cls_b1, cls_g, cls_b, cls_w2, cls_b2)]

    row = np.asarray(edge_index[0], np.int64)
    col = np.asarray(edge_index[1], np.int64)

    # ---- launch 1: edge weights ----
    if "edge" not in _CACHE:
        _CACHE["edge"] = build_edge_kernel()
    nc1 = _CACHE["edge"]
    in1 = _prep_edge_inputs(edgenet_input, en_w1, en_b1, en_g1, en_be1, en_w2, en_b2)
    r1 = bass_utils.run_bass_kernel_spmd(nc1, in1, core_ids=list(range(NCORES)))
    LAST_EXEC_NS["edge"] = r1.exec_time_ns
    LAST_RESULTS["edge"] = r1
    ew = np.concatenate([r1.results[c]["ew"].reshape(-1) for c in range(NCORES)])[:E]
    ew = ew.astype(np.float32)

    # ---- host: symmetric normalization ----
    deg = np.zeros(N, np.float32)
    np.add.at(deg, row, ew)
    dis = np.where(deg > 0, 1.0 / np.sqrt(np.maximum(deg, 1e-30)), 0.0).astype(np.float32)
    norm = (-dis[row] * ew * dis[col]).astype(np.float32)

    # ---- launch 2: GNN ----
    in2, CPB = _prep_gnn_inputs(features, row, col, norm, cheb_w,
                                cls_w1, cls_b1, cls_g, cls_b, cls_w2, cls_b2)
    key = ("gnn", CPB)
    if key not in _CACHE:
        _CACHE[key] = build_gnn_kernel(CPB)
    nc2 = _CACHE[key]
    r2 = bass_utils.run_bass_kernel_spmd(nc2, in2, core_ids=list(range(NCORES)))
    LAST_EXEC_NS["gnn"] = r2.exec_time_ns
    LAST_RESULTS["gnn"] = r2

    logit = np.concatenate(
        [r2.results[c]["logitT"].T for c in range(NCORES)], axis=0)[:N]
    return logit.astype(np.float32), ew


# revision 26
# speedup vs baseline: 1.4133x; 1.4133x over previous
"""Trainium2 Bass kernel for nn_AxisNetFusion (ChebConv GNN + PAE edge encoder).

Strategy (8 NeuronCores, node/destination-partitioned graph parallel):
  Launch 1 (edge MLP): edges split evenly across cores; each core runs the
    PAE parser (two matmul stacks sharing weights) on its edge slice and
    emits the cosine edge weights.
  Host: degree / symmetric normalization (O(E) scalar math) + builds the
    col-sorted scatter structure: per-core per-destination-block one-hot
    matrices scaled by `norm` (so the scatter is a plain PSUM-accumulated
    matmul) and int16 gather indices for the per-edge x[row] rows.
  Launch 2 (4 ChebConv layers + JK + cls head): each core owns 2560
    destination rows. Per propagation: dma_gather of x[row] rows, scatter
    via one-hot matmuls into PSUM, AllGather of the updated node slices
    between cores. Layer update h = relu(sum_k Txk @ Wk) runs in a
    transposed layout fed by PE transposes of the prop outputs.
"""

import os
import sys

sys.path.insert(0, "/opt/trn_rl_repo")

import numpy as np
import ml_dtypes
from concourse import bass, bacc, mybir, tile
from concourse import bass_utils
from concourse.masks import make_identity

F32 = mybir.dt.float32
BF16 = mybir.dt.bfloat16
NPBF16 = ml_dtypes.bfloat16
I16 = mybir.dt.int16
AF = mybir.ActivationFunctionType
OP = mybir.AluOpType

# Problem sizes (fixed by the task spec).
N = 20000
E = 400000
D = 256
L = 4
EIN = 32
EH = 128
NCLS = 2
JK = L * D
BN_EPS = 1e-5

NCORES = 8
NP = 20480            # padded node count (8 * 2560)
R = NP // NCORES      # rows (destination nodes) per core
NB = R // 128         # 128-row destination blocks per core

EPC = 50176           # padded edges per core for the edge MLP (98 * 512)
CHA = EPC // 512      # edge-MLP chunks of 512 edges
EPAD = NCORES * EPC

_CACHE: dict = {}
LAST_EXEC_NS: dict = {}
LAST_RESULTS: dict = {}


# --------------------------------------------------------------------------
# Launch 1: PAE edge encoder -> edge weights
# --------------------------------------------------------------------------
def build_edge_kernel():
    nc = bacc.Bacc(trn_type="TRN2", num_devices=NCORES)
    xT = nc.dram_tensor("xT", [2 * EIN, EPC], F32, kind="ExternalInput").ap()
    w1 = nc.dram_tensor("w1", [EIN, EH], F32, kind="ExternalInput").ap()
    w2p = nc.dram_tensor("w2p", [EH, EH], F32, kind="ExternalInput").ap()
    b1 = nc.dram_tensor("b1", [EH, 1], F32, kind="ExternalInput").ap()
    b2p = nc.dram_tensor("b2p", [EH, 1], F32, kind="ExternalInput").ap()
    tones = nc.dram_tensor("tones", [128, 256], F32, kind="ExternalInput").ap()
    ew = nc.dram_tensor("ew", [CHA, 512], F32, kind="ExternalOutput").ap()

    with tile.TileContext(nc) as tc:
        with (
            tc.tile_pool(name="const", bufs=1) as cp,
            tc.tile_pool(name="sb", bufs=3) as sb,
            tc.tile_pool(name="ps", bufs=2, space="PSUM") as ps,
            tc.tile_pool(name="pss", bufs=1, space="PSUM") as pss,
        ):
            w1_sb = cp.tile([EIN, EH], F32)
            nc.sync.dma_start(out=w1_sb[:], in_=w1[:, :])
            w2p_sb = cp.tile([EH, EH], F32)
            nc.sync.dma_start(out=w2p_sb[:], in_=w2p[:, :])
            b1_sb = cp.tile([EH, 1], F32)
            nc.sync.dma_start(out=b1_sb[:], in_=b1[:, :])
            b2p_sb = cp.tile([EH, 1], F32)
            nc.sync.dma_start(out=b2p_sb[:], in_=b2p[:, :])
            tones_sb = cp.tile([128, 256], F32)
            nc.sync.dma_start(out=tones_sb[:], in_=tones[:, :])

            # Per-edge gram terms accumulate here: rows = chunk id,
            # cols = [s11 | s12 | s22] x 512 edges.
            psS = pss.tile([128, 1536], F32, space="PSUM")

            for c in range(CHA):
                xta = sb.tile([EIN, 512], F32, tag="xta")
                nc.sync.dma_start(out=xta[:], in_=xT[0:EIN, c * 512:(c + 1) * 512])
                xtb = sb.tile([EIN, 512], F32, tag="xtb")
                nc.sync.dma_start(out=xtb[:],
                                  in_=xT[EIN:2 * EIN, c * 512:(c + 1) * 512])

                pm1 = ps.tile([128, 512], F32, tag="pm1", space="PSUM")
                nc.tensor.matmul(out=pm1[:], lhsT=w1_sb[:], rhs=xta[:],
                                 start=True, stop=True)
                r1re = sb.tile([128, 512], F32, tag="r1re")
                nc.scalar.activation(r1re[:], pm1[:], AF.Relu, bias=b1_sb[:, 0:1])

                pm1b = ps.tile([128, 512], F32, tag="pm1", space="PSUM")
                nc.tensor.matmul(out=pm1b[:], lhsT=w1_sb[:], rhs=xtb[:],
                                 start=True, stop=True)
                r2re = sb.tile([128, 512], F32, tag="r2re")
                nc.scalar.activation(r2re[:], pm1b[:], AF.Relu, bias=b1_sb[:, 0:1])

                pm2 = ps.tile([128, 512], F32, tag="pm2", space="PSUM")
                nc.tensor.matmul(out=pm2[:], lhsT=w2p_sb[:], rhs=r1re[:],
                                 start=True, stop=True)
                r1 = sb.tile([128, 512], F32, tag="r1")
                nc.vector.tensor_scalar(r1[:], pm2[:], b2p_sb[:, 0:1], None, op0=OP.add)

                pm2b = ps.tile([128, 512], F32, tag="pm2", space="PSUM")
                nc.tensor.matmul(out=pm2b[:], lhsT=w2p_sb[:], rhs=r2re[:],
                                 start=True, stop=True)
                r2 = sb.tile([128, 512], F32, tag="r2")
                nc.vector.tensor_scalar(r2[:], pm2b[:], b2p_sb[:, 0:1], None, op0=OP.add)

                prod = sb.tile([128, 1536], F32, tag="prod")
                nc.vector.tensor_tensor(prod[:, 0:512], r1[:], r1[:], op=OP.mult)
                nc.vector.tensor_tensor(prod[:, 512:1024], r1[:], r2[:], op=OP.mult)
                nc.vector.tensor_tensor(prod[:, 1024:1536], r2[:], r2[:], op=OP.mult)

                # Column-reduce (over hidden dim = partitions) into row c of psS
                # via a ones-column matmul; the sliding window of `tones` puts
                # the all-ones column at local column c.
                for j in range(3):
                    nc.tensor.matmul(
                        out=psS[:, j * 512:(j + 1) * 512],
                        lhsT=tones_sb[:, 128 - c:256 - c],
                        rhs=prod[:, j * 512:(j + 1) * 512],
                        start=(c == 0), stop=(c == CHA - 1),
                        skip_group_check=True,
                    )

            # cos = s12 / (max(sqrt(s11),eps) * max(sqrt(s22),eps));
            # ew = (cos+1)/2, all on [chunks x 512] full-width tiles.
            m11 = sb.tile([128, 512], F32, tag="m11")
            nc.vector.tensor_scalar(m11[:], psS[:, 0:512], 1e-16, None, op0=OP.max)
            m22 = sb.tile([128, 512], F32, tag="m22")
            nc.vector.tensor_scalar(m22[:], psS[:, 1024:1536], 1e-16, None, op0=OP.max)
            pm = sb.tile([128, 512], F32, tag="pm")
            nc.vector.tensor_tensor(pm[:], m11[:], m22[:], op=OP.mult)
            sq = sb.tile([128, 512], F32, tag="sq")
            nc.scalar.activation(sq[:], pm[:], AF.Sqrt)
            inv = sb.tile([128, 512], F32, tag="inv")
            nc.vector.reciprocal(inv[:], sq[:])
            cosv = sb.tile([128, 512], F32, tag="cosv")
            nc.vector.tensor_tensor(cosv[:], psS[:, 512:1024], inv[:], op=OP.mult)
            ewt = sb.tile([128, 512], F32, tag="ewt")
            nc.scalar.activation(ewt[:], cosv[:], AF.Copy, bias=0.5, scale=0.5)
            nc.sync.dma_start(out=ew[:, :], in_=ewt[0:CHA, :])

    nc.compile()
    return nc


# --------------------------------------------------------------------------
# Launch 2: 4x ChebConv + JK concat + cls head
# --------------------------------------------------------------------------
def build_gnn_kernel(CPB):
    TOTCH = NB * CPB
    nc = bacc.Bacc(trn_type="TRN2", num_devices=NCORES, num_swdge_queues=4)
    xfull0 = nc.dram_tensor("xfull0", [NP, D], BF16, kind="ExternalInput").ap()
    xown = nc.dram_tensor("xown", [R, D], F32, kind="ExternalInput").ap()
    gidx = nc.dram_tensor("gidx", [128, TOTCH * 8], I16, kind="ExternalInput").ap()
    lhsTd = nc.dram_tensor("lhsTd", [128, TOTCH * 128], BF16, kind="ExternalInput").ap()
    chebd = nc.dram_tensor("chebd", [L * 3 * D, D], F32, kind="ExternalInput").ap()
    clsw1d = nc.dram_tensor("clsw1d", [JK, D], F32, kind="ExternalInput").ap()
    clsb1d = nc.dram_tensor("clsb1d", [D, 1], F32, kind="ExternalInput").ap()
    clsw2d = nc.dram_tensor("clsw2d", [D, NCLS], F32, kind="ExternalInput").ap()
    clsb2d = nc.dram_tensor("clsb2d", [NCLS, 1], F32, kind="ExternalInput").ap()
    logitT = nc.dram_tensor("logitT", [NCLS, R], F32, kind="ExternalOutput").ap()

    RG = [list(range(NCORES))]

    with tile.TileContext(nc) as tc:
        with (
            tc.tile_pool(name="const", bufs=1) as cp,
            tc.tile_pool(name="big", bufs=1) as bigp,
            tc.tile_pool(name="gp", bufs=2) as gp,
            tc.tile_pool(name="lp", bufs=2) as lp,
            tc.tile_pool(name="wp", bufs=8) as wp,
            tc.tile_pool(name="sm", bufs=3) as sm,
            tc.tile_pool(name="ps_scat", bufs=2, space="PSUM") as ps_scat,
            tc.tile_pool(name="ps_tp", bufs=2, space="PSUM") as ps_tp,
            tc.tile_pool(name="ps_acc", bufs=2, space="PSUM") as ps_acc,
            tc.tile_pool(name="dram", bufs=1, space="DRAM") as drp,
        ):
            # ---------------- constants / persistent tiles ----------------
            gidx_sb = cp.tile([128, TOTCH * 8], I16)
            nc.sync.dma_start(out=gidx_sb[:], in_=gidx[:, :])
            ident = cp.tile([128, 128], F32)
            make_identity(nc, ident[:])
            clsb1_sb = []
            clsw2_sb = []
            for mc in range(2):
                bt = cp.tile([128, 1], F32, tag=f"clsb1_{mc}")
                nc.sync.dma_start(out=bt[:], in_=clsb1d[mc * 128:(mc + 1) * 128, :])
                clsb1_sb.append(bt)
                wt2 = cp.tile([128, NCLS], F32, tag=f"clsw2_{mc}")
                nc.sync.dma_start(out=wt2[:], in_=clsw2d[mc * 128:(mc + 1) * 128, :])
                clsw2_sb.append(wt2)
            clsb2_sb = cp.tile([NCLS, 1], F32)
            nc.sync.dma_start(out=clsb2_sb[:], in_=clsb2d[:, :])

            # 8 big [128, R] tiles: transposed Tx / h accumulators.
            bigt = [bigp.tile([128, R], F32, tag=f"big{i}", name=f"big{i}")
                    for i in range(8)]
            tx0T = [bigt[0], bigt[1]]
            tx1T = [bigt[2], bigt[3]]
            tx2T = [bigt[4], bigt[5]]
            hT = [bigt[6], bigt[7]]

            # internal DRAM
            tx1slice = drp.tile([R, D], BF16)
            hslice = drp.tile([R, D], BF16)
            tx1full = [drp.tile([NP, D], BF16, addr_space="Shared",
                                name=f"tx1full{l}", tag=f"tx1full{l}")
                       for l in range(L)]
            xf = [drp.tile([NP, D], BF16, addr_space="Shared",
                           name=f"xf{l}", tag=f"xf{l}")
                  for l in range(L - 1)]
            jkTd = drp.tile([2 * D, R], F32)  # hT of layers 0 and 1

            self_q = [0]  # SWDGE queue round-robin counter

            def scatter_blocks(xsrc_ap, post):
                """One propagation: for each destination block, gather x[row]
                rows and PSUM-accumulate one-hot matmuls; `post(b, pscat)`
                consumes the [128, D] PSUM block."""
                GSUB = 8  # <=1024 indices per dma_gather (HW packet limit)
                for b in range(NB):
                    g = gp.tile([128, CPB, D], BF16, tag="g")
                    for c0 in range(0, CPB, GSUB):
                        c1 = min(c0 + GSUB, CPB)
                        nc.gpsimd.dma_gather(
                            out_ap=g[:, c0:c1, :],
                            in_ap=xsrc_ap,
                            idxs_ap=gidx_sb[:, (b * CPB + c0) * 8:(b * CPB + c1) * 8],
                            num_idxs=(c1 - c0) * 128,
                            num_idxs_reg=(c1 - c0) * 128,
                            elem_size=D,
                            queue_num=self_q[0] % 4,
                        )
                        self_q[0] += 1
                    lt = lp.tile([128, CPB * 128], BF16, tag="lt")
                    nc.sync.dma_start(
                        out=lt[:],
                        in_=lhsTd[:, b * CPB * 128:(b + 1) * CPB * 128])
                    pscat = ps_scat.tile([128, D], F32, tag="scat", space="PSUM")
                    for cc in range(CPB):
                        nc.tensor.matmul(
                            out=pscat[:],
                            lhsT=lt[:, cc * 128:(cc + 1) * 128],
                            rhs=g[:, cc, :],
                            start=(cc == 0), stop=(cc == CPB - 1),
                            skip_group_check=True,
                        )
                    post(b, pscat)

            def transpose_into(src_sb, dstT, b):
                # src_sb [128, D] node-major block b -> dstT pair [128, R] cols
                for kc in range(2):
                    ptp = ps_tp.tile([128, 128], F32, tag="tp", space="PSUM")
                    nc.tensor.transpose(ptp[:], src_sb[:, kc * 128:(kc + 1) * 128],
                                        ident[:])
                    nc.vector.tensor_copy(dstT[kc][:, b * 128:(b + 1) * 128], ptp[:])

            for l in range(L):
                xsrc = xfull0[:, :] if l == 0 else xf[l - 1][:, :]
                agh_dst = xf[l] if l < L - 1 else None

                # ---- prop1: Tx1 = S x ----
                with nc.named_scope(f"prop1_l{l}"):
                    def post1(b, pscat):
                        t1 = sm.tile([128, D], F32, tag="t1b")
                        nc.scalar.activation(t1[:], pscat[:], AF.Copy)
                        t1h = sm.tile([128, D], BF16, tag="t1h")
                        nc.vector.tensor_copy(t1h[:], pscat[:])
                        nc.sync.dma_start(
                            out=tx1slice[b * 128:(b + 1) * 128, :], in_=t1h[:])
                        transpose_into(t1, tx1T, b)
                    scatter_blocks(xsrc, post1)

                with nc.named_scope(f"ag_tx1_l{l}"):
                    nc.gpsimd.collective_compute(
                        "AllGather", OP.bypass, replica_groups=RG,
                        ins=[tx1slice.opt()], outs=[tx1full[l].opt()])

                # ---- prop2: Tx2 = 2 S Tx1 - Tx0 ----
                with nc.named_scope(f"prop2_l{l}"):
                    def post2(b, pscat):
                        if l == 0:
                            t0b = sm.tile([128, D], F32, tag="t0b")
                            nc.sync.dma_start(
                                out=t0b[:], in_=xown[b * 128:(b + 1) * 128, :])
                        else:
                            t0h = sm.tile([128, D], BF16, tag="t0h")
                            nc.sync.dma_start(
                                out=t0h[:], in_=hslice[b * 128:(b + 1) * 128, :])
                            t0b = sm.tile([128, D], F32, tag="t0b")
                            nc.scalar.activation(t0b[:], t0h[:], AF.Copy)
                        t2a = sm.tile([128, D], F32, tag="t2a")
                        nc.scalar.activation(t2a[:], pscat[:], AF.Copy, scale=2.0)
                        t2 = sm.tile([128, D], F32, tag="t2b")
                        nc.vector.tensor_tensor(t2[:], t2a[:], t0b[:], op=OP.subtract)
                        transpose_into(t2, tx2T, b)
                        if l == 0:
                            transpose_into(t0b, tx0T, b)
                    scatter_blocks(tx1full[l][:, :], post2)

                # ---- layer update: hT = relu(sum_k Wk^T TxkT) ----
                with nc.named_scope(f"hmm_l{l}"):
                    wt = {}
                    for k in range(3):
                        for kc in range(2):
                            w = wp.tile([128, D], F32, tag="wt")
                            row0 = ((l * 3 + k) * 2 + kc) * 128
                            nc.sync.dma_start(out=w[:], in_=chebd[row0:row0 + 128, :])
                            wt[(k, kc)] = w
                    txTs = [tx0T, tx1T, tx2T]
                    for mc in range(2):
                        for nn in range(5):
                            pacc = ps_acc.tile([128, 512], F32, tag="acc",
                                               space="PSUM")
                            for k in range(3):
                                for kc in range(2):
                                    nc.tensor.matmul(
                                        out=pacc[:],
                                        lhsT=wt[(k, kc)][:, mc * 128:(mc + 1) * 128],
                                        rhs=txTs[k][kc][:, nn * 512:(nn + 1) * 512],
                                        start=(k == 0 and kc == 0),
                                        stop=(k == 2 and kc == 1),
                                        skip_group_check=True,
                                    )
                            nc.scalar.activation(
                                hT[mc][:, nn * 512:(nn + 1) * 512], pacc[:], AF.Relu)

                    if l <= 1:
                        for mc in range(2):
                            nc.sync.dma_start(
                                out=jkTd[(l * 2 + mc) * 128:(l * 2 + mc + 1) * 128, :],
                                in_=hT[mc][:, :])

                # ---- transpose h back, store slice, AllGather ----
                if l < L - 1:
                    with nc.named_scope(f"hback_l{l}"):
                        for b in range(NB):
                            hb = sm.tile([128, D], BF16, tag="hb")
                            for kc in range(2):
                                ptp = ps_tp.tile([128, 128], F32, tag="tp",
                                                 space="PSUM")
                                nc.tensor.transpose(
                                    ptp[:], hT[kc][:, b * 128:(b + 1) * 128], ident[:])
                                nc.vector.tensor_copy(
                                    hb[:, kc * 128:(kc + 1) * 128], ptp[:])
                            nc.sync.dma_start(
                                out=hslice[b * 128:(b + 1) * 128, :], in_=hb[:])
                        nc.gpsimd.collective_compute(
                            "AllGather", OP.bypass, replica_groups=RG,
                            ins=[hslice.opt()], outs=[agh_dst.opt()])

                    tx0T, hT = hT, tx0T

            # ---------------- cls head ----------------
            with nc.named_scope("cls"):
                # jk rhs tiles: layers 0/1 from DRAM, 2/3 still in SBUF
                jk_rhs = []
                for j in range(4):
                    dst = [tx1T, tx2T][j // 2][j % 2]
                    nc.sync.dma_start(out=dst[:, :], in_=jkTd[j * 128:(j + 1) * 128, :])
                    jk_rhs.append(dst)
                jk_rhs += [tx0T[0], tx0T[1], hT[0], hT[1]]

                cw = []
                for j in range(8):
                    w = wp.tile([128, D], F32, tag="cw")
                    nc.sync.dma_start(out=w[:], in_=clsw1d[j * 128:(j + 1) * 128, :])
                    cw.append(w)

                for nn in range(5):
                    plog = ps_tp.tile([NCLS, 512], F32, tag="lg", space="PSUM")
                    for mc in range(2):
                        pz = ps_acc.tile([128, 512], F32, tag="acc", space="PSUM")
                        for j in range(8):
                            nc.tensor.matmul(
                                out=pz[:],
                                lhsT=cw[j][:, mc * 128:(mc + 1) * 128],
                                rhs=jk_rhs[j][:, nn * 512:(nn + 1) * 512],
                                start=(j == 0), stop=(j == 7),
                                skip_group_check=True,
                            )
                        zr = sm.tile([128, 512], F32, tag="zr")
                        nc.scalar.activation(
                            zr[:], pz[:], AF.Relu, bias=clsb1_sb[mc][:, 0:1])
                        nc.tensor.matmul(
                            out=plog[:], lhsT=clsw2_sb[mc][:, :],
                            rhs=zr[:], start=(mc == 0), stop=(mc == 1),
                            skip_group_check=True,
                        )
                    lg = sm.tile([NCLS, 512], F32, tag="lgs")
                    nc.vector.tensor_scalar(
                        lg[:], plog[:], clsb2_sb[:, 0:1], None, op0=OP.add)
                    nc.sync.dma_start(
                        out=logitT[:, nn * 512:(nn + 1) * 512], in_=lg[:])

    nc.compile()
    return nc


# --------------------------------------------------------------------------
# Host orchestration
# --------------------------------------------------------------------------
def _wrap_idx16(vals):
    """[n] -> [128, n//16] int16 in the SWDGE wrap layout (16-partition wrap,
    replicated to all 8 Q7 partition groups)."""
    n = vals.shape[0]
    m = np.zeros((16, n // 16), np.int16)
    m[np.arange(n) % 16, np.arange(n) // 16] = vals
    return np.tile(m, (8, 1))


def _prep_edge_inputs(edgenet_input, en_w1, en_b1, en_g1, en_be1, en_w2, en_b2):
    g1k = (en_g1 / np.sqrt(np.float32(1.0 + BN_EPS))).astype(np.float32)
    w2p = (g1k[:, None] * en_w2).astype(np.float32)
    b2p = (en_be1 @ en_w2 + en_b2).astype(np.float32)
    xpad = np.zeros((EPAD, 2 * EIN), np.float32)
    xpad[:E] = edgenet_input
    tones = np.zeros((128, 256), np.float32)
    tones[:, 128] = 1.0
    in_maps = []
    for c in range(NCORES):
        xT = np.ascontiguousarray(xpad[c * EPC:(c + 1) * EPC].T)
        in_maps.append({
            "xT": xT,
            "w1": np.ascontiguousarray(en_w1),
            "w2p": w2p,
            "b1": en_b1.reshape(EH, 1).astype(np.float32),
            "b2p": b2p.reshape(EH, 1),
            "tones": tones,
        })
    return in_maps


def _prep_gnn_inputs(features, row, col, norm, cheb_w,
                     cls_w1, cls_b1, cls_g, cls_b, cls_w2, cls_b2):
    xfull0 = np.zeros((NP, D), np.float32)
    xfull0[:N] = features
    xfull0_bf = xfull0.astype(NPBF16)

    order = np.argsort(col, kind="stable")
    rs, cs, ns = row[order], col[order], norm[order].astype(np.float32)

    # per (core, block) edge counts
    blk = cs // 128                      # global block id, < 160
    counts = np.bincount(blk, minlength=NCORES * NB)
    CPB = max(1, int(np.ceil(counts.max() / 128)))
    TOTCH = NB * CPB
    starts = np.zeros(NCORES * NB + 1, np.int64)
    np.cumsum(counts, out=starts[1:])

    gk = (cls_g / np.sqrt(np.float32(1.0 + BN_EPS))).astype(np.float32)
    clsw2p = (gk[:, None] * cls_w2).astype(np.float32)
    clsb2p = (cls_b @ cls_w2 + cls_b2).astype(np.float32).reshape(NCLS, 1)
    chebd = np.ascontiguousarray(cheb_w.reshape(L * 3 * D, D))
    clsw1 = np.ascontiguousarray(cls_w1)
    clsb1 = cls_b1.reshape(D, 1).astype(np.float32)

    in_maps = []
    for c in range(NCORES):
        gv = np.zeros(TOTCH * 128, np.int64)          # gather row ids
        lh = np.zeros((128, TOTCH * 128), NPBF16)      # scaled one-hot lhsT
        for b in range(NB):
            gb = c * NB + b
            s, e = starts[gb], starts[gb + 1]
            cnt = e - s
            slot = np.arange(cnt)
            t = b * CPB + slot // 128                 # chunk id within core
            k = slot % 128                            # edge lane
            gv[t * 128 + k] = rs[s:e]
            lh[k, t * 128 + (cs[s:e] - (c * R + b * 128))] = ns[s:e].astype(NPBF16)
        in_maps.append({
            "xfull0": xfull0_bf,
            "xown": np.ascontiguousarray(xfull0[c * R:(c + 1) * R]),
            "gidx": _wrap_idx16(gv.astype(np.int16)),
            "lhsTd": lh,
            "chebd": chebd,
            "clsw1d": clsw1,
            "clsb1d": clsb1,
            "clsw2d": clsw2p,
            "clsb2d": clsb2p,
        })
    return in_maps, CPB


def kernel(features, edge_index, edgenet_input, cheb_w,
           en_w1, en_b1, en_g1, en_be1, en_w2, en_b2,
           cls_w1, cls_b1, cls_g, cls_b, cls_w2, cls_b2):
    features = np.asarray(features, np.float32)
    edge_index = np.asarray(edge_index)
    edgenet_input = np.asarray(edgenet_input, np.float32)
    cheb_w = np.asarray(cheb_w, np.float32)
    en_w1, en_b1, en_g1, en_be1, en_w2, en_b2 = [
        np.asarray(a, np.float32) for a in (en_w1, en_b1, en_g1, en_be1, en_w2, en_b2)]
    cls_w1, cls_b1, cls_g, cls_b, cls_w2, cls_b2 = [
        np.asarray(a, np.float32) for a in (cls_w1, cls_b1, cls_g, cls_b, cls_w2, cls_b2)]

    row = np.asarray(edge_index[0], np.int64)
    col = np.asarray(edge_index[1], np.int64)

    # ---- launch 1: edge weights ----
    if "edge" not in _CACHE:
        _CACHE["edge"] = build_edge_kernel()
    nc1 = _CACHE["edge"]
    in1 = _prep_edge_inputs(edgenet_input, en_w1, en_b1, en_g1, en_be1, en_w2, en_b2)
    r1 = bass_utils.run_bass_kernel_spmd(nc1, in1, core_ids=list(range(NCORES)))
    LAST_EXEC_NS["edge"] = r1.exec_time_ns
    LAST_RESULTS["edge"] = r1
    ew = np.concatenate([r1.results[c]["ew"].reshape(-1) for c in range(NCORES)])[:E]
    ew = ew.astype(np.float32)

    # ---- host: symmetric normalization ----
    deg = np.zeros(N, np.float32)
    np.add.at(deg, row, ew)
    dis = np.where(deg > 0, 1.0 / np.sqrt(np.maximum(deg, 1e-30)), 0.0).astype(np.float32)
    norm = (-dis[row] * ew * dis[col]).astype(np.float32)

    # ---- launch 2: GNN ----
    in2, CPB = _prep_gnn_inputs(features, row, col, norm, cheb_w,
                                cls_w1, cls_b1, cls_g, cls_b, cls_w2, cls_b2)
    key = ("gnn", CPB)
    if key not in _CACHE:
        _CACHE[key] = build_gnn_kernel(CPB)
    nc2 = _CACHE[key]
    r2 = bass_utils.run_bass_kernel_spmd(nc2, in2, core_ids=list(range(NCORES)))
    LAST_EXEC_NS["gnn"] = r2.exec_time_ns
    LAST_RESULTS["gnn"] = r2

    logit = np.concatenate(
        [r2.results[c]["logitT"].T for c in range(NCORES)], axis=0)[:N]
    return logit.astype(np.float32), ew


# revision 37
# speedup vs baseline: 1.5323x; 1.0842x over previous
"""Trainium2 Bass kernel for nn_AxisNetFusion (ChebConv GNN + PAE edge encoder).

Strategy (8 NeuronCores, node/destination-partitioned graph parallel):
  Launch 1 (edge MLP): edges split evenly across cores; each core runs the
    PAE parser (two matmul stacks sharing weights) on its edge slice and
    emits the cosine edge weights.
  Host: degree / symmetric normalization (O(E) scalar math) + builds the
    col-sorted scatter structure: per-core per-destination-block one-hot
    matrices scaled by `norm` (so the scatter is a plain PSUM-accumulated
    matmul) and int16 gather indices for the per-edge x[row] rows.
  Launch 2 (4 ChebConv layers + JK + cls head): each core owns 2560
    destination rows. Per propagation: dma_gather of x[row] rows, scatter
    via one-hot matmuls into PSUM, AllGather of the updated node slices
    between cores. Layer update h = relu(sum_k Txk @ Wk) runs in a
    transposed layout fed by PE transposes of the prop outputs.
"""

import os
import sys

sys.path.insert(0, "/opt/trn_rl_repo")

import numpy as np
import ml_dtypes
from concourse import bass, bacc, mybir, tile
from concourse import bass_utils
from concourse.masks import make_identity

F32 = mybir.dt.float32
BF16 = mybir.dt.bfloat16
NPBF16 = ml_dtypes.bfloat16
I16 = mybir.dt.int16
AF = mybir.ActivationFunctionType
OP = mybir.AluOpType

# Problem sizes (fixed by the task spec).
N = 20000
E = 400000
D = 256
L = 4
EIN = 32
EH = 128
NCLS = 2
JK = L * D
BN_EPS = 1e-5

NCORES = 8
NP = 20480            # padded node count (8 * 2560)
R = NP // NCORES      # rows (destination nodes) per core
NB = R // 128         # 128-row destination blocks per core

EPC = 50176           # padded edges per core for the edge MLP (98 * 512)
CHA = EPC // 512      # edge-MLP chunks of 512 edges
EPAD = NCORES * EPC

_CACHE: dict = {}
LAST_EXEC_NS: dict = {}
LAST_RESULTS: dict = {}


# --------------------------------------------------------------------------
# Launch 1: PAE edge encoder -> edge weights
# --------------------------------------------------------------------------
def build_edge_kernel():
    nc = bacc.Bacc(trn_type="TRN2", num_devices=NCORES)
    xT = nc.dram_tensor("xT", [2 * EIN, EPC], BF16, kind="ExternalInput").ap()
    w1 = nc.dram_tensor("w1", [EIN, EH], BF16, kind="ExternalInput").ap()
    w2p = nc.dram_tensor("w2p", [EH, EH], BF16, kind="ExternalInput").ap()
    b1 = nc.dram_tensor("b1", [EH, 1], F32, kind="ExternalInput").ap()
    b2p = nc.dram_tensor("b2p", [EH, 1], F32, kind="ExternalInput").ap()
    tones = nc.dram_tensor("tones", [128, 256], BF16, kind="ExternalInput").ap()
    ew = nc.dram_tensor("ew", [CHA, 512], F32, kind="ExternalOutput").ap()

    with tile.TileContext(nc) as tc:
        with (
            tc.tile_pool(name="const", bufs=1) as cp,
            tc.tile_pool(name="sb", bufs=3) as sb,
            tc.tile_pool(name="ps", bufs=2, space="PSUM") as ps,
            tc.tile_pool(name="pss", bufs=1, space="PSUM") as pss,
        ):
            w1_sb = cp.tile([EIN, EH], BF16)
            nc.sync.dma_start(out=w1_sb[:], in_=w1[:, :])
            w2p_sb = cp.tile([EH, EH], BF16)
            nc.sync.dma_start(out=w2p_sb[:], in_=w2p[:, :])
            b1_sb = cp.tile([EH, 1], F32)
            nc.sync.dma_start(out=b1_sb[:], in_=b1[:, :])
            b2p_sb = cp.tile([EH, 1], F32)
            nc.sync.dma_start(out=b2p_sb[:], in_=b2p[:, :])
            tones_sb = cp.tile([128, 256], BF16)
            nc.sync.dma_start(out=tones_sb[:], in_=tones[:, :])

            # Per-edge gram terms accumulate here: rows = chunk id,
            # cols = [s11 | s12 | s22] x 512 edges.
            psS = pss.tile([128, 1536], F32, space="PSUM")

            for c in range(CHA):
                xta = sb.tile([EIN, 512], BF16, tag="xta")
                nc.sync.dma_start(out=xta[:], in_=xT[0:EIN, c * 512:(c + 1) * 512])
                xtb = sb.tile([EIN, 512], BF16, tag="xtb")
                nc.sync.dma_start(out=xtb[:],
                                  in_=xT[EIN:2 * EIN, c * 512:(c + 1) * 512])

                pm1 = ps.tile([128, 512], F32, tag="pm1", space="PSUM")
                nc.tensor.matmul(out=pm1[:], lhsT=w1_sb[:], rhs=xta[:],
                                 start=True, stop=True)
                r1re = sb.tile([128, 512], BF16, tag="r1re")
                nc.scalar.activation(r1re[:], pm1[:], AF.Relu, bias=b1_sb[:, 0:1])

                pm1b = ps.tile([128, 512], F32, tag="pm1", space="PSUM")
                nc.tensor.matmul(out=pm1b[:], lhsT=w1_sb[:], rhs=xtb[:],
                                 start=True, stop=True)
                r2re = sb.tile([128, 512], BF16, tag="r2re")
                nc.scalar.activation(r2re[:], pm1b[:], AF.Relu, bias=b1_sb[:, 0:1])

                pm2 = ps.tile([128, 512], F32, tag="pm2", space="PSUM")
                nc.tensor.matmul(out=pm2[:], lhsT=w2p_sb[:], rhs=r1re[:],
                                 start=True, stop=True)
                r1 = sb.tile([128, 512], BF16, tag="r1")
                nc.vector.tensor_scalar(r1[:], pm2[:], b2p_sb[:, 0:1], None, op0=OP.add)

                pm2b = ps.tile([128, 512], F32, tag="pm2", space="PSUM")
                nc.tensor.matmul(out=pm2b[:], lhsT=w2p_sb[:], rhs=r2re[:],
                                 start=True, stop=True)
                r2 = sb.tile([128, 512], BF16, tag="r2")
                nc.vector.tensor_scalar(r2[:], pm2b[:], b2p_sb[:, 0:1], None, op0=OP.add)

                prod = sb.tile([128, 1536], BF16, tag="prod")
                nc.vector.tensor_tensor(prod[:, 0:512], r1[:], r1[:], op=OP.mult)
                nc.vector.tensor_tensor(prod[:, 512:1024], r1[:], r2[:], op=OP.mult)
                nc.vector.tensor_tensor(prod[:, 1024:1536], r2[:], r2[:], op=OP.mult)

                # Column-reduce (over hidden dim = partitions) into row c of psS
                # via a ones-column matmul; the sliding window of `tones` puts
                # the all-ones column at local column c.
                for j in range(3):
                    nc.tensor.matmul(
                        out=psS[:, j * 512:(j + 1) * 512],
                        lhsT=tones_sb[:, 128 - c:256 - c],
                        rhs=prod[:, j * 512:(j + 1) * 512],
                        start=(c == 0), stop=(c == CHA - 1),
                        skip_group_check=True,
                    )

            # cos = s12 / (max(sqrt(s11),eps) * max(sqrt(s22),eps));
            # ew = (cos+1)/2, all on [chunks x 512] full-width tiles.
            m11 = sb.tile([128, 512], F32, tag="m11")
            nc.vector.tensor_scalar(m11[:], psS[:, 0:512], 1e-16, None, op0=OP.max)
            m22 = sb.tile([128, 512], F32, tag="m22")
            nc.vector.tensor_scalar(m22[:], psS[:, 1024:1536], 1e-16, None, op0=OP.max)
            pm = sb.tile([128, 512], F32, tag="pm")
            nc.vector.tensor_tensor(pm[:], m11[:], m22[:], op=OP.mult)
            sq = sb.tile([128, 512], F32, tag="sq")
            nc.scalar.activation(sq[:], pm[:], AF.Sqrt)
            inv = sb.tile([128, 512], F32, tag="inv")
            nc.vector.reciprocal(inv[:], sq[:])
            cosv = sb.tile([128, 512], F32, tag="cosv")
            nc.vector.tensor_tensor(cosv[:], psS[:, 512:1024], inv[:], op=OP.mult)
            ewt = sb.tile([128, 512], F32, tag="ewt")
            nc.scalar.activation(ewt[:], cosv[:], AF.Copy, bias=0.5, scale=0.5)
            nc.sync.dma_start(out=ew[:, :], in_=ewt[0:CHA, :])

    nc.compile()
    return nc


# --------------------------------------------------------------------------
# Launch 2: 4x ChebConv + JK concat + cls head
# --------------------------------------------------------------------------
def build_gnn_kernel(CPB):
    TOTCH = NB * CPB
    nc = bacc.Bacc(trn_type="TRN2", num_devices=NCORES, num_swdge_queues=4)
    xfull0 = nc.dram_tensor("xfull0", [NP, D], BF16, kind="ExternalInput").ap()
    xown = nc.dram_tensor("xown", [R, D], F32, kind="ExternalInput").ap()
    gidx = nc.dram_tensor("gidx", [128, TOTCH * 8], I16, kind="ExternalInput").ap()
    lhsTd = nc.dram_tensor("lhsTd", [128, TOTCH * 128], BF16, kind="ExternalInput").ap()
    chebd = nc.dram_tensor("chebd", [L * 3 * D, D], F32, kind="ExternalInput").ap()
    clsw1d = nc.dram_tensor("clsw1d", [JK, D], F32, kind="ExternalInput").ap()
    clsb1d = nc.dram_tensor("clsb1d", [D, 1], F32, kind="ExternalInput").ap()
    clsw2d = nc.dram_tensor("clsw2d", [D, NCLS], F32, kind="ExternalInput").ap()
    clsb2d = nc.dram_tensor("clsb2d", [NCLS, 1], F32, kind="ExternalInput").ap()
    logitT = nc.dram_tensor("logitT", [NCLS, R], F32, kind="ExternalOutput").ap()

    RG = [list(range(NCORES))]

    with tile.TileContext(nc) as tc:
        with (
            tc.tile_pool(name="const", bufs=1) as cp,
            tc.tile_pool(name="big", bufs=1) as bigp,
            tc.tile_pool(name="gp", bufs=3) as gp,
            tc.tile_pool(name="lp", bufs=3) as lp,
            tc.tile_pool(name="wp", bufs=8) as wp,
            tc.tile_pool(name="sm", bufs=2) as sm,
            tc.tile_pool(name="ps_scat", bufs=2, space="PSUM") as ps_scat,
            tc.tile_pool(name="ps_tp", bufs=2, space="PSUM") as ps_tp,
            tc.tile_pool(name="ps_acc", bufs=2, space="PSUM") as ps_acc,
            tc.tile_pool(name="dram", bufs=1, space="DRAM") as drp,
        ):
            # ---------------- constants / persistent tiles ----------------
            gidx_sb = cp.tile([128, TOTCH * 8], I16)
            nc.sync.dma_start(out=gidx_sb[:], in_=gidx[:, :])
            ident = cp.tile([128, 128], F32)
            make_identity(nc, ident[:])
            clsb1_sb = []
            clsw2_sb = []
            for mc in range(2):
                bt = cp.tile([128, 1], F32, tag=f"clsb1_{mc}")
                nc.sync.dma_start(out=bt[:], in_=clsb1d[mc * 128:(mc + 1) * 128, :])
                clsb1_sb.append(bt)
                wt2 = cp.tile([128, NCLS], F32, tag=f"clsw2_{mc}")
                nc.sync.dma_start(out=wt2[:], in_=clsw2d[mc * 128:(mc + 1) * 128, :])
                clsw2_sb.append(wt2)
            clsb2_sb = cp.tile([NCLS, 1], F32)
            nc.sync.dma_start(out=clsb2_sb[:], in_=clsb2d[:, :])

            # 8 big [128, R] tiles: transposed Tx / h accumulators.
            bigt = [bigp.tile([128, R], F32, tag=f"big{i}", name=f"big{i}")
                    for i in range(8)]
            tx0T = [bigt[0], bigt[1]]
            tx1T = [bigt[2], bigt[3]]
            tx2T = [bigt[4], bigt[5]]
            hT = [bigt[6], bigt[7]]
            hacc = [bigp.tile([128, R], F32, tag=f"hacc{i}", name=f"hacc{i}")
                    for i in range(2)]

            # internal DRAM
            tx1slice = drp.tile([R, D], BF16)
            hslice = drp.tile([R, D], BF16)
            tx1full = [drp.tile([NP, D], BF16, addr_space="Shared",
                                name=f"tx1full{l}", tag=f"tx1full{l}")
                       for l in range(L)]
            xf = [drp.tile([NP, D], BF16, addr_space="Shared",
                           name=f"xf{l}", tag=f"xf{l}")
                  for l in range(L - 1)]
            jkTd = drp.tile([2 * D, R], F32)  # hT of layers 0 and 1

            self_q = [0]  # SWDGE queue round-robin counter

            def scatter_blocks(xsrc_ap, post):
                """One propagation: for each destination block, gather x[row]
                rows and PSUM-accumulate one-hot matmuls; `post(b, pscat)`
                consumes the [128, D] PSUM block."""
                GSUB = 8  # <=1024 indices per dma_gather (HW packet limit)
                for b in range(NB):
                    g = gp.tile([128, CPB, D], BF16, tag="g")
                    for c0 in range(0, CPB, GSUB):
                        c1 = min(c0 + GSUB, CPB)
                        nc.gpsimd.dma_gather(
                            out_ap=g[:, c0:c1, :],
                            in_ap=xsrc_ap,
                            idxs_ap=gidx_sb[:, (b * CPB + c0) * 8:(b * CPB + c1) * 8],
                            num_idxs=(c1 - c0) * 128,
                            num_idxs_reg=(c1 - c0) * 128,
                            elem_size=D,
                            queue_num=self_q[0] % 4,
                        )
                        self_q[0] += 1
                    lt = lp.tile([128, CPB * 128], BF16, tag="lt")
                    nc.sync.dma_start(
                        out=lt[:],
                        in_=lhsTd[:, b * CPB * 128:(b + 1) * CPB * 128])
                    pscat = ps_scat.tile([128, D], F32, tag="scat", space="PSUM")
                    for cc in range(CPB):
                        nc.tensor.matmul(
                            out=pscat[:],
                            lhsT=lt[:, cc * 128:(cc + 1) * 128],
                            rhs=g[:, cc, :],
                            start=(cc == 0), stop=(cc == CPB - 1),
                            skip_group_check=True,
                        )
                    post(b, pscat)

            def transpose_into(src_sb, dstT, b):
                # src_sb [128, D] node-major block b -> dstT pair [128, R] cols
                for kc in range(2):
                    ptp = ps_tp.tile([128, 128], F32, tag="tp", space="PSUM")
                    nc.tensor.transpose(ptp[:], src_sb[:, kc * 128:(kc + 1) * 128],
                                        ident[:])
                    nc.vector.tensor_copy(dstT[kc][:, b * 128:(b + 1) * 128], ptp[:])

            for l in range(L):
                xsrc = xfull0[:, :] if l == 0 else xf[l - 1][:, :]
                agh_dst = xf[l] if l < L - 1 else None

                if l == 0:
                    # build tx0T from own feature rows up front
                    with nc.named_scope("tx0prep"):
                        for b in range(NB):
                            t0p = sm.tile([128, D], F32, tag="t0p")
                            nc.sync.dma_start(
                                out=t0p[:], in_=xown[b * 128:(b + 1) * 128, :])
                            transpose_into(t0p, tx0T, b)

                # ---- prop1: Tx1 = S x ----
                with nc.named_scope(f"prop1_l{l}"):
                    def post1(b, pscat):
                        t1 = sm.tile([128, D], F32, tag="t1b")
                        nc.scalar.activation(t1[:], pscat[:], AF.Copy)
                        t1h = sm.tile([128, D], BF16, tag="t1h")
                        nc.vector.tensor_copy(t1h[:], pscat[:])
                        nc.sync.dma_start(
                            out=tx1slice[b * 128:(b + 1) * 128, :], in_=t1h[:])
                        transpose_into(t1, tx1T, b)
                    scatter_blocks(xsrc, post1)

                with nc.named_scope(f"ag_tx1_l{l}"):
                    nc.gpsimd.collective_compute(
                        "AllGather", OP.bypass, replica_groups=RG,
                        ins=[tx1slice.opt()], outs=[tx1full[l].opt()])

                # ---- partial layer update (overlaps the AllGather):
                #      hacc = W0^T Tx0T + W1^T Tx1T ----
                with nc.named_scope(f"hmmA_l{l}"):
                    wt = {}
                    for k in range(3):
                        for kc in range(2):
                            w = wp.tile([128, D], BF16 if False else F32, tag="wt",
                                        name=f"wt{l}_{k}_{kc}")
                            row0 = ((l * 3 + k) * 2 + kc) * 128
                            nc.sync.dma_start(out=w[:], in_=chebd[row0:row0 + 128, :])
                            wt[(k, kc)] = w
                    txTs = [tx0T, tx1T, tx2T]
                    for mc in range(2):
                        for nn in range(5):
                            pacc = ps_acc.tile([128, 512], F32, tag="acc",
                                               space="PSUM")
                            for k in range(2):
                                for kc in range(2):
                                    nc.tensor.matmul(
                                        out=pacc[:],
                                        lhsT=wt[(k, kc)][:, mc * 128:(mc + 1) * 128],
                                        rhs=txTs[k][kc][:, nn * 512:(nn + 1) * 512],
                                        start=(k == 0 and kc == 0),
                                        stop=(k == 1 and kc == 1),
                                        skip_group_check=True,
                                    )
                            nc.scalar.activation(
                                hacc[mc][:, nn * 512:(nn + 1) * 512], pacc[:],
                                AF.Copy)

                # ---- prop2: Tx2 = 2 S Tx1 - Tx0 ----
                with nc.named_scope(f"prop2_l{l}"):
                    def post2(b, pscat):
                        if l == 0:
                            t0b = sm.tile([128, D], F32, tag="t0b")
                            nc.sync.dma_start(
                                out=t0b[:], in_=xown[b * 128:(b + 1) * 128, :])
                        else:
                            t0h = sm.tile([128, D], BF16, tag="t0h")
                            nc.sync.dma_start(
                                out=t0h[:], in_=hslice[b * 128:(b + 1) * 128, :])
                            t0b = sm.tile([128, D], F32, tag="t0b")
                            nc.scalar.activation(t0b[:], t0h[:], AF.Copy)
                        t2a = sm.tile([128, D], F32, tag="t2a")
                        nc.scalar.activation(t2a[:], pscat[:], AF.Copy, scale=2.0)
                        t2 = sm.tile([128, D], F32, tag="t2b")
                        nc.vector.tensor_tensor(t2[:], t2a[:], t0b[:], op=OP.subtract)
                        transpose_into(t2, tx2T, b)
                    scatter_blocks(tx1full[l][:, :], post2)

                # ---- final layer update: hT = relu(hacc + W2^T Tx2T) ----
                with nc.named_scope(f"hmm_l{l}"):
                    for mc in range(2):
                        for nn in range(5):
                            pacc = ps_acc.tile([128, 512], F32, tag="acc",
                                               space="PSUM")
                            for kc in range(2):
                                nc.tensor.matmul(
                                    out=pacc[:],
                                    lhsT=wt[(2, kc)][:, mc * 128:(mc + 1) * 128],
                                    rhs=tx2T[kc][:, nn * 512:(nn + 1) * 512],
                                    start=(kc == 0), stop=(kc == 1),
                                    skip_group_check=True,
                                )
                            hsum = sm.tile([128, 512], F32, tag="hsum")
                            nc.vector.tensor_tensor(
                                hsum[:], pacc[:],
                                hacc[mc][:, nn * 512:(nn + 1) * 512], op=OP.add)
                            nc.scalar.activation(
                                hT[mc][:, nn * 512:(nn + 1) * 512], hsum[:], AF.Relu)

                    if l <= 1:
                        for mc in range(2):
                            nc.sync.dma_start(
                                out=jkTd[(l * 2 + mc) * 128:(l * 2 + mc + 1) * 128, :],
                                in_=hT[mc][:, :])

                # ---- transpose h back, store slice, AllGather ----
                if l < L - 1:
                    with nc.named_scope(f"hback_l{l}"):
                        for b in range(NB):
                            hb = sm.tile([128, D], BF16, tag="hb")
                            for kc in range(2):
                                ptp = ps_tp.tile([128, 128], F32, tag="tp",
                                                 space="PSUM")
                                nc.tensor.transpose(
                                    ptp[:], hT[kc][:, b * 128:(b + 1) * 128], ident[:])
                                nc.vector.tensor_copy(
                                    hb[:, kc * 128:(kc + 1) * 128], ptp[:])
                            nc.sync.dma_start(
                                out=hslice[b * 128:(b + 1) * 128, :], in_=hb[:])
                        nc.gpsimd.collective_compute(
                            "AllGather", OP.bypass, replica_groups=RG,
                            ins=[hslice.opt()], outs=[agh_dst.opt()])

                    tx0T, hT = hT, tx0T

            # ---------------- cls head ----------------
            with nc.named_scope("cls"):
                # jk rhs tiles: layers 0/1 from DRAM, 2/3 still in SBUF
                jk_rhs = []
                for j in range(4):
                    dst = [tx1T, tx2T][j // 2][j % 2]
                    nc.sync.dma_start(out=dst[:, :], in_=jkTd[j * 128:(j + 1) * 128, :])
                    jk_rhs.append(dst)
                jk_rhs += [tx0T[0], tx0T[1], hT[0], hT[1]]

                cw = []
                for j in range(8):
                    w = wp.tile([128, D], F32, tag="cw")
                    nc.sync.dma_start(out=w[:], in_=clsw1d[j * 128:(j + 1) * 128, :])
                    cw.append(w)

                for nn in range(5):
                    plog = ps_tp.tile([NCLS, 512], F32, tag="lg", space="PSUM")
                    for mc in range(2):
                        pz = ps_acc.tile([128, 512], F32, tag="acc", space="PSUM")
                        for j in range(8):
                            nc.tensor.matmul(
                                out=pz[:],
                                lhsT=cw[j][:, mc * 128:(mc + 1) * 128],
                                rhs=jk_rhs[j][:, nn * 512:(nn + 1) * 512],
                                start=(j == 0), stop=(j == 7),
                                skip_group_check=True,
                            )
                        zr = sm.tile([128, 512], F32, tag="zr")
                        nc.scalar.activation(
                            zr[:], pz[:], AF.Relu, bias=clsb1_sb[mc][:, 0:1])
                        nc.tensor.matmul(
                            out=plog[:], lhsT=clsw2_sb[mc][:, :],
                            rhs=zr[:], start=(mc == 0), stop=(mc == 1),
                            skip_group_check=True,
                        )
                    lg = sm.tile([NCLS, 512], F32, tag="lgs")
                    nc.vector.tensor_scalar(
                        lg[:], plog[:], clsb2_sb[:, 0:1], None, op0=OP.add)
                    nc.sync.dma_start(
                        out=logitT[:, nn * 512:(nn + 1) * 512], in_=lg[:])

    nc.compile()
    return nc


# --------------------------------------------------------------------------
# Host orchestration
# --------------------------------------------------------------------------
def _wrap_idx16(vals):
    """[n] -> [128, n//16] int16 in the SWDGE wrap layout (16-partition wrap,
    replicated to all 8 Q7 partition groups)."""
    n = vals.shape[0]
    m = np.zeros((16, n // 16), np.int16)
    m[np.arange(n) % 16, np.arange(n) // 16] = vals
    return np.tile(m, (8, 1))


def _prep_edge_inputs(edgenet_input, en_w1, en_b1, en_g1, en_be1, en_w2, en_b2):
    g1k = (en_g1 / np.sqrt(np.float32(1.0 + BN_EPS))).astype(np.float32)
    w2p = (g1k[:, None] * en_w2).astype(np.float32)
    b2p = (en_be1 @ en_w2 + en_b2).astype(np.float32)
    xpad = np.zeros((EPAD, 2 * EIN), np.float32)
    xpad[:E] = edgenet_input
    xpadT = np.ascontiguousarray(xpad.T.astype(NPBF16))
    tones = np.zeros((128, 256), NPBF16)
    tones[:, 128] = 1.0
    in_maps = []
    for c in range(NCORES):
        in_maps.append({
            "xT": np.ascontiguousarray(xpadT[:, c * EPC:(c + 1) * EPC]),
            "w1": np.ascontiguousarray(en_w1.astype(NPBF16)),
            "w2p": w2p.astype(NPBF16),
            "b1": en_b1.reshape(EH, 1).astype(np.float32),
            "b2p": b2p.reshape(EH, 1),
            "tones": tones,
        })
    return in_maps


def _prep_gnn_inputs(features, row, col, norm, cheb_w,
                     cls_w1, cls_b1, cls_g, cls_b, cls_w2, cls_b2):
    xfull0 = np.zeros((NP, D), np.float32)
    xfull0[:N] = features
    xfull0_bf = xfull0.astype(NPBF16)

    order = np.argsort(col, kind="stable")
    rs, cs, ns = row[order], col[order], norm[order].astype(np.float32)

    # per (core, block) edge counts
    blk = cs // 128                      # global block id, < 160
    counts = np.bincount(blk, minlength=NCORES * NB)
    CPB = max(1, int(np.ceil(counts.max() / 128)))
    TOTCH = NB * CPB
    starts = np.zeros(NCORES * NB + 1, np.int64)
    np.cumsum(counts, out=starts[1:])

    gk = (cls_g / np.sqrt(np.float32(1.0 + BN_EPS))).astype(np.float32)
    clsw2p = (gk[:, None] * cls_w2).astype(np.float32)
    clsb2p = (cls_b @ cls_w2 + cls_b2).astype(np.float32).reshape(NCLS, 1)
    chebd = np.ascontiguousarray(cheb_w.reshape(L * 3 * D, D))
    clsw1 = np.ascontiguousarray(cls_w1)
    clsb1 = cls_b1.reshape(D, 1).astype(np.float32)

    in_maps = []
    for c in range(NCORES):
        gv = np.zeros(TOTCH * 128, np.int64)          # gather row ids
        lh = np.zeros((128, TOTCH * 128), NPBF16)      # scaled one-hot lhsT
        for b in range(NB):
            gb = c * NB + b
            s, e = starts[gb], starts[gb + 1]
            cnt = e - s
            slot = np.arange(cnt)
            t = b * CPB + slot // 128                 # chunk id within core
            k = slot % 128                            # edge lane
            gv[t * 128 + k] = rs[s:e]
            lh[k, t * 128 + (cs[s:e] - (c * R + b * 128))] = ns[s:e].astype(NPBF16)
        in_maps.append({
            "xfull0": xfull0_bf,
            "xown": np.ascontiguousarray(xfull0[c * R:(c + 1) * R]),
            "gidx": _wrap_idx16(gv.astype(np.int16)),
            "lhsTd": lh,
            "chebd": chebd,
            "clsw1d": clsw1,
            "clsb1d": clsb1,
            "clsw2d": clsw2p,
            "clsb2d": clsb2p,
        })
    return in_maps, CPB


def kernel(features, edge_index, edgenet_input, cheb_w,
           en_w1, en_b1, en_g1, en_be1, en_w2, en_b2,
           cls_w1, cls_b1, cls_g, cls_b, cls_w2, cls_b2):
    features = np.asarray(features, np.float32)
    edge_index = np.asarray(edge_index)
    edgenet_input = np.asarray(edgenet_input, np.float32)
    cheb_w = np.asarray(cheb_w, np.float32)
    en_w1, en_b1, en_g1, en_be1, en_w2, en_b2 = [
        np.asarray(a, np.float32) for a in (en_w1, en_b1, en_g1, en_be1, en_w2, en_b2)]
    cls_w1, cls_b1, cls_g, cls_b, cls_w2, cls_b2 = [
        np.asarray(a, np.float32) for a in (cls_w1, cls_b1, cls_g, cls_b, cls_w2, cls_b2)]

    row = np.asarray(edge_index[0], np.int64)
    col = np.asarray(edge_index[1], np.int64)

    # ---- launch 1: edge weights ----
    if "edge" not in _CACHE:
        _CACHE["edge"] = build_edge_kernel()
    nc1 = _CACHE["edge"]
    in1 = _prep_edge_inputs(edgenet_input, en_w1, en_b1, en_g1, en_be1, en_w2, en_b2)
    r1 = bass_utils.run_bass_kernel_spmd(nc1, in1, core_ids=list(range(NCORES)))
    LAST_EXEC_NS["edge"] = r1.exec_time_ns
    LAST_RESULTS["edge"] = r1
    ew = np.concatenate([r1.results[c]["ew"].reshape(-1) for c in range(NCORES)])[:E]
    ew = ew.astype(np.float32)

    # ---- host: symmetric normalization ----
    deg = np.zeros(N, np.float32)
    np.add.at(deg, row, ew)
    dis = np.where(deg > 0, 1.0 / np.sqrt(np.maximum(deg, 1e-30)), 0.0).astype(np.float32)
    norm = (-dis[row] * ew * dis[col]).astype(np.float32)

    # ---- launch 2: GNN ----
    in2, CPB = _prep_gnn_inputs(features, row, col, norm, cheb_w,
                                cls_w1, cls_b1, cls_g, cls_b, cls_w2, cls_b2)
    key = ("gnn", CPB)
    if key not in _CACHE:
        _CACHE[key] = build_gnn_kernel(CPB)
    nc2 = _CACHE[key]
    r2 = bass_utils.run_bass_kernel_spmd(nc2, in2, core_ids=list(range(NCORES)))
    LAST_EXEC_NS["gnn"] = r2.exec_time_ns
    LAST_RESULTS["gnn"] = r2

    logit = np.concatenate(
        [r2.results[c]["logitT"].T for c in range(NCORES)], axis=0)[:N]
    return logit.astype(np.float32), ew
